# revision 1
# baseline (speedup 1.0000x reference)
"""CPL loss (all-support) Trainium2 kernel - no-collective SPMD design.

Math reformulation
------------------
Reference: for each query q, gather S=91 sample queries (90 negatives drawn per
class via a fixed jax PRNG + the query itself), compute cosine similarity of the
20 supports of q's class against the 91 samples, log-softmax over samples, NLL
at the self position, mean over (q, k), then an extra 1/nq.

Every sample is itself one of the 1000 queries, so all needed cosine
similarities are entries of the support x query Gram matrix ``Ghat``. With
``Ehat = exp(Ghat)`` the per-(support,query) softmax denominator is

    SumExp[r, q] = sum_{s in samples(q)} Ehat[r, s] = (Ehat @ Mask)[r, q]

where Mask[q', q] counts occurrences of query q' in q's sample multiset (host
precomputed - it depends only on the PRNG + labels, not on float data).

    loss = (Sum2 - Sum1) / (nq * K * nq)
    Sum1 = sum_{q,k} Ghat[20*lbl(q)+k, q]          (target logits)
    Sum2 = sum_{q,k} log(SumExp[20*lbl(q)+k, q])   (log denominators)

Sharding (no collectives - measured cost of ANY collective in this runtime is
~65us of barrier/skew/ncfw overhead, far more than the redundant compute it
saves): queries are label-sorted and sharded 8 x 125. A 125-query window of
the sorted order touches few labels (2 for the block-labeled episodic layout),
so core j only needs the Gram slab of its shard's `20*n_lab` support rows
against ALL 1000 queries (sample multisets span every query). Each core
computes its slab, both norm sets, the mask matmul over the full contraction,
and its own (Sum1_j, Sum2_j) partials; the host sums 8 partial pairs and
applies the constants (the unshard/gather step).

Per-core device pipeline (matmul inputs bf16, accumulation fp32):
  1. ssq_q via ACT/DVE squares of the d-major query tiles + a PE ones-matmul
     reduction -> (1,1000); transposed to per-partition chunks by tiny K=1
     matmuls; 1/sqrt via ACT Sqrt + DVE reciprocal. Same for the supports.
  2. slab = S_lab^T-tiles x Q^T-tiles -> psum (nsl x 1000) fp32.
  3. per 128-column chunk: PE transpose -> (128 x nsl), scale by support norms
     (broadcast tile), Exp(scale=query-norm) on ACT -> Ehat^T chunk (bf16);
     masked target-term accumulation on DVE for Sum1.
  4. mask matmul: 8 chunk matmuls accumulate psum (nsl x 125) = SumExp for
     this shard's own 125 query columns.
  5. Ln on ACT, row-ownership mask, reductions -> (Sum1_j, Sum2_j) -> DRAM.
"""

import os
import numpy as np
import ml_dtypes

import concourse.bass as bass
import concourse.mybir as mybir
import concourse.tile as tile
from concourse import bass_utils
from concourse.vector_clock import ScopedClock

N_WAY = 10
Q_PER = 100
K_SHOT = 20
D = 2048
M_NEG = 10
NQ = N_WAY * Q_PER          # 1000
NS = N_WAY * K_SHOT         # 200
S_SAMP = (N_WAY - 1) * M_NEG + 1  # 91
N_CORES = 8
QSH = NQ // N_CORES         # 125
KT = D // 128               # 16
NB = 512                    # psum bank f32 capacity (column split)
NCH = (NQ + 127) // 128     # 8 query chunks
ACT_SQ_TILES = 6            # square tiles 0..5 on ACT, rest on DVE

F32 = mybir.dt.float32
BF16 = mybir.dt.bfloat16
BF16_NP = ml_dtypes.bfloat16

_last_exec_time_ns = None
_last_results = None


def _mk_wait(nc, engine, w):
    wi = mybir.InstEventSemaphore(name=nc.get_next_instruction_name(), engine=engine)
    wi.sync_info = mybir.SyncInfo(on_wait=[w], on_update=[])
    return wi


class _TileContextSplitWaits(tile.TileContext):
    """Workaround for a walrus build that rejects >1 sync-wait per
    instruction: peel extra waits onto standalone single-wait EventSemaphore
    instructions on the same (in-order) engine queue."""

    def _add_instruction(self, inst):
        si = inst.sync_info
        if si is not None and si.on_wait and len(si.on_wait) > 1:
            waits = list(si.on_wait)
            for w in waits[:-1]:
                super()._add_instruction(_mk_wait(self.nc, inst.engine, w))
            si.on_wait = waits[-1:]
        super()._add_instruction(inst)

    def _drain_and_barrier(self, tick_clock, wait_clock):
        nc = self.nc
        drain_inst = nc.sync.drain()
        wait_clock.add_sem_waits(
            drain_inst.ins, ScopedClock({None: tick_clock.global_clock})
        )
        si = drain_inst.ins.sync_info
        waits = list(si.on_wait or [])
        if len(waits) > 1:
            si.on_wait = waits[:1]
            for w in waits[1:]:
                self._add_instruction(_mk_wait(nc, drain_inst.ins.engine, w))

        nc.all_engine_barrier()
        assert self.sems is not None
        popped = nc._tile_sem_poison_stack.pop()
        assert popped is self._sem_poison
        nc.clear_and_free_semaphores(list(self.sems.allocated().values()))
        nc.all_engine_barrier()


def _sample_idx(labels_query: np.ndarray) -> np.ndarray:
    """Replicate the reference's per-query negative sampling exactly."""
    import jax
    import jax.numpy as jnp

    cpu = jax.devices("cpu")[0]
    with jax.default_device(cpu):
        key = jax.random.key(42)
        u = jax.random.uniform(key, (NQ, N_WAY, Q_PER))
        _, topm = jax.lax.top_k(u, M_NEG)
        lbl = jnp.asarray(labels_query).astype(jnp.int32)
        j = jnp.arange(N_WAY - 1)
        other = j[None, :] + (j[None, :] >= lbl[:, None])
        sel = jnp.take_along_axis(topm, other[:, :, None], axis=1)
        neg_idx = (other[:, :, None] * Q_PER + sel).reshape(NQ, -1)
        sample_idx = jnp.concatenate([neg_idx, jnp.arange(NQ)[:, None]], axis=1)
        return np.asarray(sample_idx)


def _tileize_dT(mat_t: np.ndarray, ncols: int, dtype) -> np.ndarray:
    """(D, ncols) -> (128, KT*ncols): free slice k*ncols:(k+1)*ncols is the
    k-th 128-row chunk of the D-major matrix."""
    return np.ascontiguousarray(
        mat_t.reshape(KT, 128, ncols).transpose(1, 0, 2).reshape(128, KT * ncols)
    ).astype(dtype)


def _tileize_rows(mat: np.ndarray, width: int, dtype) -> np.ndarray:
    """(NQ, width) -> (128, NCH*width): free slice c*width:(c+1)*width is rows
    [128c, 128c+128) (zero-padded past NQ)."""
    padded = np.zeros((NCH * 128, width), mat.dtype)
    padded[:NQ] = mat
    return np.ascontiguousarray(
        padded.reshape(NCH, 128, width).transpose(1, 0, 2).reshape(128, NCH * width)
    ).astype(dtype)


QSPLITS = [6, 5, 4, 1]           # uneven qt DMA pieces (k-tiles per piece)
QBOUND = [0, 6, 11, 15, 16]
ACT_SQ = {0, 1, 2, 3, 4, 5}      # early square tiles on ACT, late ones on DVE


def _build_program(n_lab: int):
    """Build the SPMD Bass program (identical on all 8 cores)."""
    nsl = K_SHOT * n_lab  # slab rows (40 for block labels)
    nc = bass.Bass("TRN2", num_devices=N_CORES)

    # bf16 inputs: bfc1 = [ st | ident ], bfc2 = [ mask | rmask ]
    B1_W = KT * nsl + nsl
    B2_W = NCH * QSH + NCH * nsl
    bfc1_d = nc.dram_tensor("bfc1", [128, B1_W], BF16, kind="ExternalInput")
    bfc2_d = nc.dram_tensor("bfc2", [128, B2_W], BF16, kind="ExternalInput")
    qt_d = nc.dram_tensor("qt", [128, KT * NQ], BF16, kind="ExternalInput")
    aux_d = nc.dram_tensor("aux", [nsl, QSH], F32, kind="ExternalInput")
    out_d = nc.dram_tensor("out", [1, 2], F32, kind="ExternalOutput")

    with _TileContextSplitWaits(nc) as tc:
        with (
            tc.tile_pool(name="sb", bufs=1) as sb,
            tc.tile_pool(name="ps", bufs=1, space="PSUM") as ps,
            tc.tile_pool(name="pst", bufs=2, space="PSUM") as pst,
        ):
            ones_col = sb.tile([128, 1], BF16, tag="ones_col")
            nc.gpsimd.memset(ones_col[:], 1.0)
            ones_row = sb.tile([1, 128], F32, tag="ones_row")
            nc.gpsimd.memset(ones_row[:], 1.0)
            ones_col_f = sb.tile([128, 1], F32, tag="ones_col_f")
            nc.gpsimd.memset(ones_col_f[:], 1.0)
            dummy = sb.tile([128, NB], BF16, tag="dummy")
            nc.gpsimd.memset(dummy[:], 1.0)

            # PE prewarm: throwaway matmuls to flip the HAM clock gate to 8/8
            # and keep it there until the first qt piece lands
            ps_warm = pst.tile([1, NB], F32, tag="scr")
            for i in range(24):
                nc.tensor.matmul(
                    ps_warm[:], ones_col[:], dummy[:], start=True, stop=True
                )

            # DMA order: first qt piece -> small st -> rest of qt -> masks
            qt = sb.tile([128, KT * NQ], BF16, tag="qt")
            w0 = QBOUND[1] * NQ
            nc.sync.dma_start(qt[:, 0:w0], qt_d[:, 0:w0])
            bfc1 = sb.tile([128, B1_W], BF16, tag="bfc1")
            nc.sync.dma_start(bfc1[:], bfc1_d[:, :])
            st = bfc1[:, 0 : KT * nsl]
            ident = bfc1[0:nsl, KT * nsl : B1_W]
            for s in range(1, 4):
                lo, hi = QBOUND[s] * NQ, QBOUND[s + 1] * NQ
                nc.sync.dma_start(qt[:, lo:hi], qt_d[:, lo:hi])
            bfc2 = sb.tile([128, B2_W], BF16, tag="bfc2")
            nc.sync.dma_start(bfc2[:], bfc2_d[:, :])
            maskt = bfc2[:, 0 : NCH * QSH]
            rmask = bfc2[:, NCH * QSH : B2_W]
            aux = sb.tile([nsl, QSH], F32, tag="aux")
            nc.sync.dma_start(aux[:], aux_d[:, :])
            rowm = aux[:, :]

            # ---- support norm squares + ones-reduction (Ln/Exp come after
            # the ACT squares to keep the table switches off-chain) ----
            st2 = sb.tile([128, KT * nsl], BF16, tag="st2")
            nc.vector.tensor_tensor(st2[:], st, st, mybir.AluOpType.mult)
            ps_sn = pst.tile([1, nsl], F32, tag="scr")
            for k in range(KT):
                nc.tensor.matmul(
                    ps_sn[:],
                    ones_col[:],
                    st2[:, k * nsl : (k + 1) * nsl],
                    start=(k == 0),
                    stop=(k == KT - 1),
                )

            # ---- per qt piece: squares (DVE/GpSimd) + slab & ssq matmuls ----
            qt2 = sb.tile([128, KT * NQ], BF16, tag="qt2")
            ps_slab = ps.tile([nsl, NQ], F32, tag="ps_slab")
            ps_ssq = ps.tile([1, NQ], F32, tag="ps_ssq")
            for s in range(4):
                for k in range(QBOUND[s], QBOUND[s + 1]):
                    src = qt[:, k * NQ : (k + 1) * NQ]
                    dst = qt2[:, k * NQ : (k + 1) * NQ]
                    if k in ACT_SQ:
                        nc.scalar.activation(
                            dst, src, mybir.ActivationFunctionType.Square
                        )
                    else:
                        nc.vector.tensor_tensor(dst, src, src, mybir.AluOpType.mult)
                for lo, hi in ((0, NB), (NB, NQ)):
                    for k in range(QBOUND[s], QBOUND[s + 1]):
                        nc.tensor.matmul(
                            ps_ssq[:, lo:hi],
                            ones_col[:],
                            qt2[:, k * NQ + lo : k * NQ + hi],
                            start=(k == 0),
                            stop=(k == KT - 1),
                            skip_group_check=True,
                        )
                for lo, hi in ((0, NB), (NB, NQ)):
                    for k in range(QBOUND[s], QBOUND[s + 1]):
                        nc.tensor.matmul(
                            ps_slab[:, lo:hi],
                            st[:, k * nsl : (k + 1) * nsl],
                            qt[:, k * NQ + lo : k * NQ + hi],
                            start=(k == 0),
                            stop=(k == KT - 1),
                            skip_group_check=True,
                        )

            # prefetch the Exp/Ln table while the PE stream finishes
            dln = sb.tile([1, 1], F32, tag="dln")
            nc.scalar.activation(
                dln[:], ones_row[0:1, 0:1], mybir.ActivationFunctionType.Ln
            )
            # deferred support-norm tail: a_s = exp(-0.5 ln(ssq_s)), broadcast
            sn_ln = sb.tile([1, nsl], F32, tag="sn_ln")
            nc.scalar.activation(
                sn_ln[:], ps_sn[:], mybir.ActivationFunctionType.Ln
            )
            sn_i = sb.tile([1, nsl], F32, tag="sn_i")
            nc.scalar.activation(
                sn_i[:], sn_ln[:], mybir.ActivationFunctionType.Exp, scale=-0.5
            )
            ps_abc = pst.tile([128, nsl], F32, tag="scr")
            nc.tensor.matmul(ps_abc[:], ones_row[:], sn_i[:], start=True, stop=True)
            a_bc = sb.tile([128, nsl], BF16, tag="a_bc")
            nc.scalar.copy(a_bc[:], ps_abc[:])

            # ---- query inverse norms: copy ssq row (bf16), tiny transpose
            # matmuls, crep = exp(-0.5 ln(.)) ----
            srow = sb.tile([1, NQ], BF16, tag="srow")
            nc.scalar.copy(srow[:, 0:NB], ps_ssq[:, 0:NB])
            nc.scalar.copy(srow[:, NB:NQ], ps_ssq[:, NB:NQ])
            ps_cq = pst.tile([128, NCH], F32, tag="scr")
            nc.vector.memset(ps_cq[:], 1.0)
            for c in range(NCH):
                pn = 128 if (c + 1) * 128 <= NQ else NQ - c * 128
                nc.tensor.matmul(
                    ps_cq[0:pn, c : c + 1],
                    srow[:, c * 128 : c * 128 + pn],
                    ones_col[0:1, 0:1],
                    start=True,
                    stop=True,
                    skip_group_check=True,
                )
            cq_ln = sb.tile([128, NCH], F32, tag="cq_ln")
            nc.scalar.activation(
                cq_ln[:], ps_cq[:], mybir.ActivationFunctionType.Ln
            )
            crep = sb.tile([128, NCH], BF16, tag="crep")
            nc.scalar.activation(
                crep[:], cq_ln[:], mybir.ActivationFunctionType.Exp, scale=-0.5
            )

            # ---- slab -> sbuf (bf16, zero-padded), transposes ----
            gs = sb.tile([nsl, NCH * 128], BF16, tag="gs")
            nc.gpsimd.memset(gs[:, NQ : NCH * 128], 0.0)
            nc.vector.tensor_copy(gs[:, 0:NB], ps_slab[:, 0:NB])
            nc.vector.tensor_copy(gs[:, NB:NQ], ps_slab[:, NB:NQ])

            ps_tall = ps.tile([128, NCH * nsl], BF16, tag="ps_tall")
            for c in range(NCH):
                nc.tensor.transpose(
                    ps_tall[:, c * nsl : (c + 1) * nsl],
                    gs[:, c * 128 : (c + 1) * 128],
                    ident,
                )

            tmp_all = sb.tile([128, NCH * nsl], BF16, tag="tmp_all")
            nc.vector.tensor_tensor(
                tmp_all[:].rearrange("p (c r) -> p c r", c=NCH),
                ps_tall[:].rearrange("p (c r) -> p c r", c=NCH),
                a_bc[:].unsqueeze(1).broadcast_to((128, NCH, nsl)),
                mybir.AluOpType.mult,
            )
            ghat_all = sb.tile([128, NCH * nsl], BF16, tag="ghat_all")
            nc.vector.tensor_tensor(
                ghat_all[:].rearrange("p (c r) -> p c r", c=NCH),
                tmp_all[:].rearrange("p (c r) -> p c r", c=NCH),
                crep[:].unsqueeze(2).broadcast_to((128, NCH, nsl)),
                mybir.AluOpType.mult,
            )
            ehat = sb.tile([128, NCH * nsl], BF16, tag="ehat")
            nc.scalar.activation(
                ehat[:], ghat_all[:], mybir.ActivationFunctionType.Exp
            )

            # ---- mask matmul: SumExp for this shard's own 125 columns ----
            ps_sum = ps.tile([nsl, QSH], F32, tag="ps_sum")
            for c in range(NCH):
                pn = 128 if (c + 1) * 128 <= NQ else NQ - c * 128
                nc.tensor.matmul(
                    ps_sum[:],
                    ehat[0:pn, c * nsl : (c + 1) * nsl],
                    maskt[0:pn, c * QSH : (c + 1) * QSH],
                    start=(c == 0),
                    stop=(c == NCH - 1),
                )

            # ---- Sum2: log + row-ownership mask + reduce ----
            lgt = sb.tile([nsl, QSH], F32, tag="lgt")
            nc.scalar.activation(lgt[:], ps_sum[:], mybir.ActivationFunctionType.Ln)
            lmskd = sb.tile([nsl, QSH], F32, tag="lmskd")
            nc.vector.tensor_tensor(lmskd[:], lgt[:], rowm, mybir.AluOpType.mult)
            v2 = sb.tile([nsl, 1], F32, tag="v2")
            nc.vector.reduce_sum(v2[:], lmskd[:], axis=mybir.AxisListType.X)

            # Sum1 partial: masked reduce of ghat over the whole strip
            mskd = sb.tile([128, NCH * nsl], BF16, tag="mskd")
            nc.vector.tensor_tensor(
                mskd[:], ghat_all[:], rmask, mybir.AluOpType.mult
            )
            v_acc = sb.tile([128, 1], F32, tag="v_acc")
            nc.vector.reduce_sum(
                v_acc[:],
                mskd[:].rearrange("p (c r) -> p c r", c=NCH),
                axis=mybir.AxisListType.XY,
            )

            ps_s1 = pst.tile([1, 1], F32, tag="scr")
            nc.tensor.matmul(ps_s1[:], v_acc[:], ones_col_f[:], start=True, stop=True)
            ps_s2 = pst.tile([1, 1], F32, tag="scr")
            nc.tensor.matmul(
                ps_s2[:], v2[:], ones_col_f[0:nsl, :], start=True, stop=True
            )

            outt = sb.tile([1, 2], F32, tag="outt")
            nc.scalar.copy(outt[:, 0:1], ps_s1[:])
            nc.scalar.copy(outt[:, 1:2], ps_s2[:])
            nc.sync.dma_start(out_d[:, :], outt[:])

    return nc


def kernel(support_set, queries, labels_query, labels_support):
    global _last_exec_time_ns, _last_results

    support_set = np.ascontiguousarray(np.asarray(support_set, dtype=np.float32))
    queries = np.ascontiguousarray(np.asarray(queries, dtype=np.float32))
    lbl = np.asarray(labels_query).astype(np.int64)

    # ---- host-side index prep (PRNG + labels only; no float math) ----
    sample_idx = _sample_idx(lbl.astype(np.int32))          # (NQ, 91)
    order = np.argsort(lbl, kind="stable")                  # sorted-query order
    pos = np.empty(NQ, dtype=np.int64)
    pos[order] = np.arange(NQ)
    lbl_sorted = lbl[order]

    # per-core label sets, padded to a common size for SPMD uniformity
    core_labs = []
    for j in range(N_CORES):
        labs = sorted(set(lbl_sorted[j * QSH : (j + 1) * QSH].tolist()))
        core_labs.append(labs)
    n_lab = max(len(l) for l in core_labs)
    for labs in core_labs:
        while len(labs) < n_lab:
            labs.append(labs[0])
    nsl = K_SHOT * n_lab

    # full sample-count matrix in sorted coordinates
    samp_pos = pos[sample_idx[order]]                        # (NQ, 91)
    mask_full = np.zeros((NQ, NQ), dtype=np.float32)
    np.add.at(
        mask_full,
        (samp_pos.ravel(), np.repeat(np.arange(NQ), S_SAMP)),
        1.0,
    )

    queries_sorted_T = np.ascontiguousarray(queries[order].T)  # (D, NQ)
    qt_tiled = _tileize_dT(queries_sorted_T, NQ, BF16_NP)

    in_maps = []
    for j in range(N_CORES):
        sl = slice(j * QSH, (j + 1) * QSH)
        labs = core_labs[j]
        sup_rows = np.concatenate(
            [np.arange(L * K_SHOT, (L + 1) * K_SHOT) for L in labs]
        )
        st_j = support_set[sup_rows]                         # (nsl, D)
        # slab-local base row of each label (first occurrence; pads excluded)
        row_of = {}
        for i, L in enumerate(labs):
            if L not in row_of:
                row_of[L] = i * K_SHOT

        # rmask: (q'_sorted, slab_row) ones at own-shard target entries
        rmask_full = np.zeros((NQ, nsl), dtype=np.float32)
        qs = np.arange(j * QSH, (j + 1) * QSH)
        base = np.array([row_of[L] for L in lbl_sorted[sl]])
        rmask_full[qs[:, None], base[:, None] + np.arange(K_SHOT)[None, :]] = 1.0

        # rowm: (slab_row, own_col) ones at the label rows of each column
        rowm = np.zeros((nsl, QSH), dtype=np.float32)
        rows2 = base[:, None] + np.arange(K_SHOT)[None, :]   # (QSH, 20)
        cols2 = np.broadcast_to(np.arange(QSH)[:, None], rows2.shape)
        rowm[rows2.ravel(), cols2.ravel()] = 1.0

        st_tiled = _tileize_dT(np.ascontiguousarray(st_j.T), nsl, BF16_NP)
        mask_tiled = _tileize_rows(mask_full[:, sl], QSH, BF16_NP)
        bfc1 = np.zeros((128, KT * nsl + nsl), dtype=BF16_NP)
        bfc1[:, 0 : KT * nsl] = st_tiled
        bfc1[0:nsl, KT * nsl :] = np.eye(nsl, dtype=np.float32).astype(BF16_NP)
        bfc2 = np.zeros((128, NCH * QSH + NCH * nsl), dtype=BF16_NP)
        bfc2[:, 0 : NCH * QSH] = mask_tiled
        bfc2[:, NCH * QSH :] = _tileize_rows(rmask_full, nsl, BF16_NP)
        in_maps.append(
            {"qt": qt_tiled, "bfc1": bfc1, "bfc2": bfc2, "aux": rowm}
        )

    nc = _build_program(n_lab)
    trace = os.environ.get("KERNEL_TRACE", "0") == "1"
    if trace:
        _enable_tracing()
    res = bass_utils.run_bass_kernel_spmd(
        nc, in_maps, core_ids=list(range(N_CORES)), trace=trace
    )
    _last_exec_time_ns = res.exec_time_ns
    _last_results = res

    parts = np.stack([res.results[j]["out"][0] for j in range(N_CORES)])  # (8, 2)
    sum1 = np.float32(parts[:, 0].sum(dtype=np.float64))
    sum2 = np.float32(parts[:, 1].sum(dtype=np.float64))
    loss = (sum2 - sum1) / np.float32(NQ * K_SHOT) / np.float32(NQ)
    return np.asarray(loss, dtype=np.float32)


def _enable_tracing():
    """Best-effort NTFF profiling under axon: install the missing
    antenv.axon_hooks shim + skip the artifact upload."""
    import sys
    import types

    if "antenv.axon_hooks" not in sys.modules:
        mod = types.ModuleType("antenv.axon_hooks")
        mod._hook = None

        def set_axon_ntff_profile_hook(h):
            mod._hook = h

        def get_axon_ntff_profile_hook():
            return mod._hook

        mod.set_axon_ntff_profile_hook = set_axon_ntff_profile_hook
        mod.get_axon_ntff_profile_hook = get_axon_ntff_profile_hook
        sys.modules["antenv.axon_hooks"] = mod
        try:
            from trn_agent_boot.trn_boot import _ntff_profile_via_ctypes

            mod._hook = _ntff_profile_via_ctypes("/opt/axon/libaxon_pjrt.so")
        except Exception as e:
            print("tracing hook unavailable:", e)
    bass_utils.upload_artifacts = lambda tmpdir: "local://skipped"



# revision 2
# speedup vs baseline: 1.0145x; 1.0145x over previous
"""CPL loss (all-support) Trainium2 kernel - no-collective SPMD design.

Math reformulation
------------------
Reference: for each query q, gather S=91 sample queries (90 negatives drawn per
class via a fixed jax PRNG + the query itself), compute cosine similarity of the
20 supports of q's class against the 91 samples, log-softmax over samples, NLL
at the self position, mean over (q, k), then an extra 1/nq.

Every sample is itself one of the 1000 queries, so all needed cosine
similarities are entries of the support x query Gram matrix ``Ghat``. With
``Ehat = exp(Ghat)`` the per-(support,query) softmax denominator is

    SumExp[r, q] = sum_{s in samples(q)} Ehat[r, s] = (Ehat @ Mask)[r, q]

where Mask[q', q] counts occurrences of query q' in q's sample multiset (host
precomputed - it depends only on the PRNG + labels, not on float data).

    loss = (Sum2 - Sum1) / (nq * K * nq)
    Sum1 = sum_{q,k} Ghat[20*lbl(q)+k, q]          (target logits)
    Sum2 = sum_{q,k} log(SumExp[20*lbl(q)+k, q])   (log denominators)

Sharding (no collectives - measured cost of ANY collective in this runtime is
~65us of barrier/skew/ncfw overhead, far more than the redundant compute it
saves): queries are label-sorted and sharded 8 x 125. A 125-query window of
the sorted order touches few labels (2 for the block-labeled episodic layout),
so core j only needs the Gram slab of its shard's `20*n_lab` support rows
against ALL 1000 queries (sample multisets span every query). Each core
computes its slab, both norm sets, the mask matmul over the full contraction,
and its own (Sum1_j, Sum2_j) partials; the host sums 8 partial pairs and
applies the constants (the unshard/gather step).

Per-core device pipeline (matmul inputs bf16, accumulation fp32):
  1. ssq_q via ACT/DVE squares of the d-major query tiles + a PE ones-matmul
     reduction -> (1,1000); transposed to per-partition chunks by tiny K=1
     matmuls; 1/sqrt via ACT Sqrt + DVE reciprocal. Same for the supports.
  2. slab = S_lab^T-tiles x Q^T-tiles -> psum (nsl x 1000) fp32.
  3. per 128-column chunk: PE transpose -> (128 x nsl), scale by support norms
     (broadcast tile), Exp(scale=query-norm) on ACT -> Ehat^T chunk (bf16);
     masked target-term accumulation on DVE for Sum1.
  4. mask matmul: 8 chunk matmuls accumulate psum (nsl x 125) = SumExp for
     this shard's own 125 query columns.
  5. Ln on ACT, row-ownership mask, reductions -> (Sum1_j, Sum2_j) -> DRAM.
"""

import os
import numpy as np
import ml_dtypes

import concourse.bass as bass
import concourse.mybir as mybir
import concourse.tile as tile
from concourse import bass_utils
from concourse.vector_clock import ScopedClock

N_WAY = 10
Q_PER = 100
K_SHOT = 20
D = 2048
M_NEG = 10
NQ = N_WAY * Q_PER          # 1000
NS = N_WAY * K_SHOT         # 200
S_SAMP = (N_WAY - 1) * M_NEG + 1  # 91
N_CORES = 8
QSH = NQ // N_CORES         # 125
KT = D // 128               # 16
NB = 512                    # psum bank f32 capacity (column split)
NCH = (NQ + 127) // 128     # 8 query chunks
ACT_SQ_TILES = 6            # square tiles 0..5 on ACT, rest on DVE

F32 = mybir.dt.float32
BF16 = mybir.dt.bfloat16
BF16_NP = ml_dtypes.bfloat16

_last_exec_time_ns = None
_last_results = None


def _mk_wait(nc, engine, w):
    wi = mybir.InstEventSemaphore(name=nc.get_next_instruction_name(), engine=engine)
    wi.sync_info = mybir.SyncInfo(on_wait=[w], on_update=[])
    return wi


class _TileContextSplitWaits(tile.TileContext):
    """Workaround for a walrus build that rejects >1 sync-wait per
    instruction: peel extra waits onto standalone single-wait EventSemaphore
    instructions on the same (in-order) engine queue."""

    def _add_instruction(self, inst):
        si = inst.sync_info
        if si is not None and si.on_wait and len(si.on_wait) > 1:
            waits = list(si.on_wait)
            for w in waits[:-1]:
                super()._add_instruction(_mk_wait(self.nc, inst.engine, w))
            si.on_wait = waits[-1:]
        super()._add_instruction(inst)

    def _drain_and_barrier(self, tick_clock, wait_clock):
        # Teardown trim: skip the end-of-program semaphore clears (the bass
        # preamble re-clears the whole kernel sem range on every execution,
        # so a one-shot NEFF never observes stale values) and the second
        # all-engine barrier.
        nc = self.nc
        drain_inst = nc.sync.drain()
        wait_clock.add_sem_waits(
            drain_inst.ins, ScopedClock({None: tick_clock.global_clock})
        )
        si = drain_inst.ins.sync_info
        waits = list(si.on_wait or [])
        if len(waits) > 1:
            si.on_wait = waits[:1]
            for w in waits[1:]:
                self._add_instruction(_mk_wait(nc, drain_inst.ins.engine, w))

        nc.all_engine_barrier()
        assert self.sems is not None
        popped = nc._tile_sem_poison_stack.pop()
        assert popped is self._sem_poison
        nc._state.prepend_free_semaphores(
            [s.num for s in self.sems.allocated().values()]
        )


def _sample_idx(labels_query: np.ndarray) -> np.ndarray:
    """Replicate the reference's per-query negative sampling exactly."""
    import jax
    import jax.numpy as jnp

    cpu = jax.devices("cpu")[0]
    with jax.default_device(cpu):
        key = jax.random.key(42)
        u = jax.random.uniform(key, (NQ, N_WAY, Q_PER))
        _, topm = jax.lax.top_k(u, M_NEG)
        lbl = jnp.asarray(labels_query).astype(jnp.int32)
        j = jnp.arange(N_WAY - 1)
        other = j[None, :] + (j[None, :] >= lbl[:, None])
        sel = jnp.take_along_axis(topm, other[:, :, None], axis=1)
        neg_idx = (other[:, :, None] * Q_PER + sel).reshape(NQ, -1)
        sample_idx = jnp.concatenate([neg_idx, jnp.arange(NQ)[:, None]], axis=1)
        return np.asarray(sample_idx)


def _tileize_dT(mat_t: np.ndarray, ncols: int, dtype) -> np.ndarray:
    """(D, ncols) -> (128, KT*ncols): free slice k*ncols:(k+1)*ncols is the
    k-th 128-row chunk of the D-major matrix."""
    return np.ascontiguousarray(
        mat_t.reshape(KT, 128, ncols).transpose(1, 0, 2).reshape(128, KT * ncols)
    ).astype(dtype)


def _tileize_rows(mat: np.ndarray, width: int, dtype) -> np.ndarray:
    """(NQ, width) -> (128, NCH*width): free slice c*width:(c+1)*width is rows
    [128c, 128c+128) (zero-padded past NQ)."""
    padded = np.zeros((NCH * 128, width), mat.dtype)
    padded[:NQ] = mat
    return np.ascontiguousarray(
        padded.reshape(NCH, 128, width).transpose(1, 0, 2).reshape(128, NCH * width)
    ).astype(dtype)


QSPLITS = [6, 5, 4, 1]           # uneven qt DMA pieces (k-tiles per piece)
QBOUND = [0, 6, 11, 15, 16]
ACT_SQ = {0, 1, 2, 3, 4, 5}      # early square tiles on ACT, late ones on DVE


def _build_program(n_lab: int):
    """Build the SPMD Bass program (identical on all 8 cores)."""
    nsl = K_SHOT * n_lab  # slab rows (40 for block labels)
    nc = bass.Bass("TRN2", num_devices=N_CORES)

    # bf16 inputs: bfc1 = [ st | ident ], bfc2 = [ mask | rmask ]
    B1_W = KT * nsl + nsl
    B2_W = NCH * QSH + NCH * nsl
    bfc1_d = nc.dram_tensor("bfc1", [128, B1_W], BF16, kind="ExternalInput")
    bfc2_d = nc.dram_tensor("bfc2", [128, B2_W], BF16, kind="ExternalInput")
    qt_d = nc.dram_tensor("qt", [128, KT * NQ], BF16, kind="ExternalInput")
    aux_d = nc.dram_tensor("aux", [nsl, QSH], F32, kind="ExternalInput")
    out_d = nc.dram_tensor("out", [1, 2], F32, kind="ExternalOutput")

    with _TileContextSplitWaits(nc) as tc:
        with (
            tc.tile_pool(name="sb", bufs=1) as sb,
            tc.tile_pool(name="ps", bufs=1, space="PSUM") as ps,
            tc.tile_pool(name="pst", bufs=2, space="PSUM") as pst,
        ):
            ones_col = sb.tile([128, 1], BF16, tag="ones_col")
            nc.gpsimd.memset(ones_col[:], 1.0)
            ones_row = sb.tile([1, 128], F32, tag="ones_row")
            nc.gpsimd.memset(ones_row[:], 1.0)
            ones_col_f = sb.tile([128, 1], F32, tag="ones_col_f")
            nc.gpsimd.memset(ones_col_f[:], 1.0)
            dummy = sb.tile([128, NB], BF16, tag="dummy")
            nc.gpsimd.memset(dummy[:], 1.0)

            # PE prewarm: throwaway matmuls to flip the HAM clock gate to 8/8
            # and keep it there until the first qt piece lands
            ps_warm = pst.tile([1, NB], F32, tag="scr")
            for i in range(24):
                nc.tensor.matmul(
                    ps_warm[:], ones_col[:], dummy[:], start=True, stop=True
                )

            # DMA order: first qt piece -> small st -> rest of qt -> masks
            qt = sb.tile([128, KT * NQ], BF16, tag="qt")
            w0 = QBOUND[1] * NQ
            nc.sync.dma_start(qt[:, 0:w0], qt_d[:, 0:w0])
            bfc1 = sb.tile([128, B1_W], BF16, tag="bfc1")
            nc.sync.dma_start(bfc1[:], bfc1_d[:, :])
            st = bfc1[:, 0 : KT * nsl]
            ident = bfc1[0:nsl, KT * nsl : B1_W]
            for s in range(1, 4):
                lo, hi = QBOUND[s] * NQ, QBOUND[s + 1] * NQ
                nc.sync.dma_start(qt[:, lo:hi], qt_d[:, lo:hi])
            bfc2 = sb.tile([128, B2_W], BF16, tag="bfc2")
            nc.sync.dma_start(bfc2[:], bfc2_d[:, :])
            maskt = bfc2[:, 0 : NCH * QSH]
            rmask = bfc2[:, NCH * QSH : B2_W]
            aux = sb.tile([nsl, QSH], F32, tag="aux")
            nc.sync.dma_start(aux[:], aux_d[:, :])
            rowm = aux[:, :]

            # ---- support norm squares + ones-reduction (Ln/Exp come after
            # the ACT squares to keep the table switches off-chain) ----
            st2 = sb.tile([128, KT * nsl], BF16, tag="st2")
            nc.vector.tensor_tensor(st2[:], st, st, mybir.AluOpType.mult)
            ps_sn = pst.tile([1, nsl], F32, tag="scr")
            for k in range(KT):
                nc.tensor.matmul(
                    ps_sn[:],
                    ones_col[:],
                    st2[:, k * nsl : (k + 1) * nsl],
                    start=(k == 0),
                    stop=(k == KT - 1),
                )

            # ---- per qt piece: squares (DVE/GpSimd) + slab & ssq matmuls ----
            qt2 = sb.tile([128, KT * NQ], BF16, tag="qt2")
            ps_slab = ps.tile([nsl, NQ], F32, tag="ps_slab")
            ps_ssq = ps.tile([1, NQ], F32, tag="ps_ssq")
            for s in range(4):
                for k in range(QBOUND[s], QBOUND[s + 1]):
                    src = qt[:, k * NQ : (k + 1) * NQ]
                    dst = qt2[:, k * NQ : (k + 1) * NQ]
                    if k in ACT_SQ:
                        nc.scalar.activation(
                            dst, src, mybir.ActivationFunctionType.Square
                        )
                    else:
                        nc.vector.tensor_tensor(dst, src, src, mybir.AluOpType.mult)
                for lo, hi in ((0, NB), (NB, NQ)):
                    for k in range(QBOUND[s], QBOUND[s + 1]):
                        nc.tensor.matmul(
                            ps_ssq[:, lo:hi],
                            ones_col[:],
                            qt2[:, k * NQ + lo : k * NQ + hi],
                            start=(k == 0),
                            stop=(k == KT - 1),
                            skip_group_check=True,
                        )
                for lo, hi in ((0, NB), (NB, NQ)):
                    for k in range(QBOUND[s], QBOUND[s + 1]):
                        nc.tensor.matmul(
                            ps_slab[:, lo:hi],
                            st[:, k * nsl : (k + 1) * nsl],
                            qt[:, k * NQ + lo : k * NQ + hi],
                            start=(k == 0),
                            stop=(k == KT - 1),
                            skip_group_check=True,
                        )

            # prefetch the Exp/Ln table while the PE stream finishes
            dln = sb.tile([1, 1], F32, tag="dln")
            nc.scalar.activation(
                dln[:], ones_row[0:1, 0:1], mybir.ActivationFunctionType.Ln
            )
            # deferred support-norm tail: a_s = exp(-0.5 ln(ssq_s)), broadcast
            sn_ln = sb.tile([1, nsl], F32, tag="sn_ln")
            nc.scalar.activation(
                sn_ln[:], ps_sn[:], mybir.ActivationFunctionType.Ln
            )
            sn_i = sb.tile([1, nsl], F32, tag="sn_i")
            nc.scalar.activation(
                sn_i[:], sn_ln[:], mybir.ActivationFunctionType.Exp, scale=-0.5
            )
            ps_abc = pst.tile([128, nsl], F32, tag="scr")
            nc.tensor.matmul(ps_abc[:], ones_row[:], sn_i[:], start=True, stop=True)
            a_bc = sb.tile([128, nsl], BF16, tag="a_bc")
            nc.scalar.copy(a_bc[:], ps_abc[:])

            # ---- query inverse norms: copy ssq row (bf16), tiny transpose
            # matmuls, crep = exp(-0.5 ln(.)) ----
            srow = sb.tile([1, NQ], BF16, tag="srow")
            nc.scalar.copy(srow[:, 0:NB], ps_ssq[:, 0:NB])
            nc.scalar.copy(srow[:, NB:NQ], ps_ssq[:, NB:NQ])
            ps_cq = pst.tile([128, NCH], F32, tag="scr")
            nc.vector.memset(ps_cq[:], 1.0)
            for c in range(NCH):
                pn = 128 if (c + 1) * 128 <= NQ else NQ - c * 128
                nc.tensor.matmul(
                    ps_cq[0:pn, c : c + 1],
                    srow[:, c * 128 : c * 128 + pn],
                    ones_col[0:1, 0:1],
                    start=True,
                    stop=True,
                    skip_group_check=True,
                )
            cq_ln = sb.tile([128, NCH], F32, tag="cq_ln")
            nc.scalar.activation(
                cq_ln[:], ps_cq[:], mybir.ActivationFunctionType.Ln
            )
            crep = sb.tile([128, NCH], BF16, tag="crep")
            nc.scalar.activation(
                crep[:], cq_ln[:], mybir.ActivationFunctionType.Exp, scale=-0.5
            )

            # ---- slab -> sbuf (bf16, zero-padded), transposes ----
            gs = sb.tile([nsl, NCH * 128], BF16, tag="gs")
            nc.gpsimd.memset(gs[:, NQ : NCH * 128], 0.0)
            nc.vector.tensor_copy(gs[:, 0:NB], ps_slab[:, 0:NB])
            nc.vector.tensor_copy(gs[:, NB:NQ], ps_slab[:, NB:NQ])

            ps_tall = ps.tile([128, NCH * nsl], BF16, tag="ps_tall")
            for c in range(NCH):
                nc.tensor.transpose(
                    ps_tall[:, c * nsl : (c + 1) * nsl],
                    gs[:, c * 128 : (c + 1) * 128],
                    ident,
                )

            tmp_all = sb.tile([128, NCH * nsl], BF16, tag="tmp_all")
            nc.vector.tensor_tensor(
                tmp_all[:].rearrange("p (c r) -> p c r", c=NCH),
                ps_tall[:].rearrange("p (c r) -> p c r", c=NCH),
                a_bc[:].unsqueeze(1).broadcast_to((128, NCH, nsl)),
                mybir.AluOpType.mult,
            )
            ghat_all = sb.tile([128, NCH * nsl], BF16, tag="ghat_all")
            nc.vector.tensor_tensor(
                ghat_all[:].rearrange("p (c r) -> p c r", c=NCH),
                tmp_all[:].rearrange("p (c r) -> p c r", c=NCH),
                crep[:].unsqueeze(2).broadcast_to((128, NCH, nsl)),
                mybir.AluOpType.mult,
            )
            ehat = sb.tile([128, NCH * nsl], BF16, tag="ehat")
            nc.scalar.activation(
                ehat[:], ghat_all[:], mybir.ActivationFunctionType.Exp
            )

            # ---- mask matmul: SumExp for this shard's own 125 columns ----
            ps_sum = ps.tile([nsl, QSH], F32, tag="ps_sum")
            for c in range(NCH):
                pn = 128 if (c + 1) * 128 <= NQ else NQ - c * 128
                nc.tensor.matmul(
                    ps_sum[:],
                    ehat[0:pn, c * nsl : (c + 1) * nsl],
                    maskt[0:pn, c * QSH : (c + 1) * QSH],
                    start=(c == 0),
                    stop=(c == NCH - 1),
                )

            # ---- Sum2: log + row-ownership mask + reduce ----
            lgt = sb.tile([nsl, QSH], F32, tag="lgt")
            nc.scalar.activation(lgt[:], ps_sum[:], mybir.ActivationFunctionType.Ln)
            lmskd = sb.tile([nsl, QSH], F32, tag="lmskd")
            nc.vector.tensor_tensor(lmskd[:], lgt[:], rowm, mybir.AluOpType.mult)
            v2 = sb.tile([nsl, 1], F32, tag="v2")
            nc.vector.reduce_sum(v2[:], lmskd[:], axis=mybir.AxisListType.X)

            # Sum1 partial: masked reduce of ghat over the whole strip
            mskd = sb.tile([128, NCH * nsl], BF16, tag="mskd")
            nc.vector.tensor_tensor(
                mskd[:], ghat_all[:], rmask, mybir.AluOpType.mult
            )
            v_acc = sb.tile([128, 1], F32, tag="v_acc")
            nc.vector.reduce_sum(
                v_acc[:],
                mskd[:].rearrange("p (c r) -> p c r", c=NCH),
                axis=mybir.AxisListType.XY,
            )

            ps_s1 = pst.tile([1, 1], F32, tag="scr")
            nc.tensor.matmul(ps_s1[:], v_acc[:], ones_col_f[:], start=True, stop=True)
            ps_s2 = pst.tile([1, 1], F32, tag="scr")
            nc.tensor.matmul(
                ps_s2[:], v2[:], ones_col_f[0:nsl, :], start=True, stop=True
            )

            outt = sb.tile([1, 2], F32, tag="outt")
            nc.scalar.copy(outt[:, 0:1], ps_s1[:])
            nc.scalar.copy(outt[:, 1:2], ps_s2[:])
            nc.sync.dma_start(out_d[:, :], outt[:])

    return nc


def kernel(support_set, queries, labels_query, labels_support):
    global _last_exec_time_ns, _last_results

    support_set = np.ascontiguousarray(np.asarray(support_set, dtype=np.float32))
    queries = np.ascontiguousarray(np.asarray(queries, dtype=np.float32))
    lbl = np.asarray(labels_query).astype(np.int64)

    # ---- host-side index prep (PRNG + labels only; no float math) ----
    sample_idx = _sample_idx(lbl.astype(np.int32))          # (NQ, 91)
    order = np.argsort(lbl, kind="stable")                  # sorted-query order
    pos = np.empty(NQ, dtype=np.int64)
    pos[order] = np.arange(NQ)
    lbl_sorted = lbl[order]

    # per-core label sets, padded to a common size for SPMD uniformity
    core_labs = []
    for j in range(N_CORES):
        labs = sorted(set(lbl_sorted[j * QSH : (j + 1) * QSH].tolist()))
        core_labs.append(labs)
    n_lab = max(len(l) for l in core_labs)
    for labs in core_labs:
        while len(labs) < n_lab:
            labs.append(labs[0])
    nsl = K_SHOT * n_lab

    # full sample-count matrix in sorted coordinates
    samp_pos = pos[sample_idx[order]]                        # (NQ, 91)
    mask_full = np.zeros((NQ, NQ), dtype=np.float32)
    np.add.at(
        mask_full,
        (samp_pos.ravel(), np.repeat(np.arange(NQ), S_SAMP)),
        1.0,
    )

    queries_sorted_T = np.ascontiguousarray(queries[order].T)  # (D, NQ)
    qt_tiled = _tileize_dT(queries_sorted_T, NQ, BF16_NP)

    in_maps = []
    for j in range(N_CORES):
        sl = slice(j * QSH, (j + 1) * QSH)
        labs = core_labs[j]
        sup_rows = np.concatenate(
            [np.arange(L * K_SHOT, (L + 1) * K_SHOT) for L in labs]
        )
        st_j = support_set[sup_rows]                         # (nsl, D)
        # slab-local base row of each label (first occurrence; pads excluded)
        row_of = {}
        for i, L in enumerate(labs):
            if L not in row_of:
                row_of[L] = i * K_SHOT

        # rmask: (q'_sorted, slab_row) ones at own-shard target entries
        rmask_full = np.zeros((NQ, nsl), dtype=np.float32)
        qs = np.arange(j * QSH, (j + 1) * QSH)
        base = np.array([row_of[L] for L in lbl_sorted[sl]])
        rmask_full[qs[:, None], base[:, None] + np.arange(K_SHOT)[None, :]] = 1.0

        # rowm: (slab_row, own_col) ones at the label rows of each column
        rowm = np.zeros((nsl, QSH), dtype=np.float32)
        rows2 = base[:, None] + np.arange(K_SHOT)[None, :]   # (QSH, 20)
        cols2 = np.broadcast_to(np.arange(QSH)[:, None], rows2.shape)
        rowm[rows2.ravel(), cols2.ravel()] = 1.0

        st_tiled = _tileize_dT(np.ascontiguousarray(st_j.T), nsl, BF16_NP)
        mask_tiled = _tileize_rows(mask_full[:, sl], QSH, BF16_NP)
        bfc1 = np.zeros((128, KT * nsl + nsl), dtype=BF16_NP)
        bfc1[:, 0 : KT * nsl] = st_tiled
        bfc1[0:nsl, KT * nsl :] = np.eye(nsl, dtype=np.float32).astype(BF16_NP)
        bfc2 = np.zeros((128, NCH * QSH + NCH * nsl), dtype=BF16_NP)
        bfc2[:, 0 : NCH * QSH] = mask_tiled
        bfc2[:, NCH * QSH :] = _tileize_rows(rmask_full, nsl, BF16_NP)
        in_maps.append(
            {"qt": qt_tiled, "bfc1": bfc1, "bfc2": bfc2, "aux": rowm}
        )

    nc = _build_program(n_lab)
    trace = os.environ.get("KERNEL_TRACE", "0") == "1"
    if trace:
        _enable_tracing()
    res = bass_utils.run_bass_kernel_spmd(
        nc, in_maps, core_ids=list(range(N_CORES)), trace=trace
    )
    _last_exec_time_ns = res.exec_time_ns
    _last_results = res

    parts = np.stack([res.results[j]["out"][0] for j in range(N_CORES)])  # (8, 2)
    sum1 = np.float32(parts[:, 0].sum(dtype=np.float64))
    sum2 = np.float32(parts[:, 1].sum(dtype=np.float64))
    loss = (sum2 - sum1) / np.float32(NQ * K_SHOT) / np.float32(NQ)
    return np.asarray(loss, dtype=np.float32)


def _enable_tracing():
    """Best-effort NTFF profiling under axon: install the missing
    antenv.axon_hooks shim + skip the artifact upload."""
    import sys
    import types

    if "antenv.axon_hooks" not in sys.modules:
        mod = types.ModuleType("antenv.axon_hooks")
        mod._hook = None

        def set_axon_ntff_profile_hook(h):
            mod._hook = h

        def get_axon_ntff_profile_hook():
            return mod._hook

        mod.set_axon_ntff_profile_hook = set_axon_ntff_profile_hook
        mod.get_axon_ntff_profile_hook = get_axon_ntff_profile_hook
        sys.modules["antenv.axon_hooks"] = mod
        try:
            from trn_agent_boot.trn_boot import _ntff_profile_via_ctypes

            mod._hook = _ntff_profile_via_ctypes("/opt/axon/libaxon_pjrt.so")
        except Exception as e:
            print("tracing hook unavailable:", e)
    bass_utils.upload_artifacts = lambda tmpdir: "local://skipped"

